# revision 1
# baseline (speedup 1.0000x reference)
"""Trainium2 kernel for nn_Dense_RBS_state_vector.

Math: each RBS gate on the Hamming-weight-2 basis is the second exterior
power (compound matrix) of a 32x32 Givens rotation. The sequential scan of
62 gates collapses to a single dense [496,496] matrix:

    M_total = Lambda^2(R_61 @ ... @ R_0),
    M_total[(a,b),(c,d)] = R[a,c]*R[b,d] - R[a,d]*R[b,c]

so the whole reference computation is one matmul: out = state @ M_total.T.
R_total / M_total are tiny (0.5 MFLOP) and computed on the host in float64
from the runtime angles.

Device kernel: batched matmul y = x @ W, data-parallel over 8 NeuronCores
(512 batch rows per core). The contraction dim must sit on SBUF partitions
for TensorE, so the host ships x pre-transposed (layout marshalling).

Precision: "f32r3" (default) — exact hi/lo fp32r decomposition computed
on device (copy-to-f32r rounds; subtract leaves the exact residual);
3 f32r matmul terms at 1 cyc/row reconstruct full fp32 accuracy:
HW-measured rel err 2.48e-7, identical to the "fp32" variant (4 cyc/row)
at ~1.55x the speed. Other flags: "fp32" (plain), "bf16x3" (~1e-5),
"f32r" (single-term TF32-like, 1.5e-4).

Per-core structure: 4-8 input DMAs (chunk-padded 3D access patterns land
all l-chunks of x^T and W in one SBUF tile each, interleaved so the first
matmuls start ~2us in) -> accumulating matmuls per 128-row batch tile
(4 l-chunk K-steps into one PSUM bank; group completions staggered) ->
PSUM->SBUF copies (DVE/ACT alternating) -> per-tile output DMAs that
pipeline behind the remaining matmuls.
"""

import numpy as np

N_QUBITS = 32
D = 496  # C(32, 2)
BATCH = 4096
NCORES = 8
BS = BATCH // NCORES  # 512
GATES = [(i, i + 1) for i in range(N_QUBITS - 1)] * 2  # 62 gates
LCH = [128, 128, 128, 112]  # l-chunk partition sizes (sum = 496)
LOF = [0, 128, 256, 384]

DEFAULT_PRECISION = "f32r3"

_NC = {}  # precision -> compiled bass module


def _host_weight(angles: np.ndarray) -> np.ndarray:
    """M_total.T [l, k] in float64 on host, from the runtime angles."""
    R = np.eye(N_QUBITS, dtype=np.float64)
    for (i, j), th in zip(GATES, np.asarray(angles, dtype=np.float64)):
        c, s = np.cos(th), np.sin(th)
        Ri, Rj = R[i].copy(), R[j].copy()
        R[i] = c * Ri + s * Rj
        R[j] = -s * Ri + c * Rj
    pairs = [(a, b) for a in range(N_QUBITS) for b in range(a + 1, N_QUBITS)]
    A = np.asarray(pairs)
    a_, b_ = A[:, 0], A[:, 1]
    M = R[np.ix_(a_, a_)] * R[np.ix_(b_, b_)] - R[np.ix_(a_, b_)] * R[np.ix_(b_, a_)]
    return np.ascontiguousarray(M.T)  # [l, k], float64


def _chunk_pad(a: np.ndarray) -> np.ndarray:
    """[496, m] -> [4, 128, m]: l-chunks of 128/128/128/112, zero-padded."""
    out = np.zeros((4, 128, a.shape[1]), dtype=a.dtype)
    for j in range(4):
        out[j, : LCH[j]] = a[LOF[j] : LOF[j] + LCH[j]]
    return out


def _bf16_pair(a32: np.ndarray):
    """fp32 -> (hi, lo) bf16 arrays."""
    import ml_dtypes

    hi = a32.astype(ml_dtypes.bfloat16)
    lo = (a32 - hi.astype(np.float32)).astype(ml_dtypes.bfloat16)
    return hi, lo


def _build_module(precision=DEFAULT_PRECISION):
    import concourse.bacc as bacc
    import concourse.mybir as mybir
    from concourse.tile import TileContext

    nc = bacc.Bacc("TRN2", target_bir_lowering=False, debug=False)
    bf16 = mybir.dt.bfloat16
    f32 = mybir.dt.float32
    f32r = mybir.dt.float32r
    NT = BS // 128  # 4 batch tiles per core

    split = precision == "bf16x3"
    in_dt = bf16 if split else (f32r if precision == "f32r" else f32)
    nh = 2 if split else 1  # number of input halves

    # xt: transposed x, chunk-padded: [nh*4, 128, BS] flattened to 2D
    xt = nc.dram_tensor("xt", [nh * 4 * 128, BS], in_dt, kind="ExternalInput").ap()
    # w: chunk-padded: [nh*4, 128, D] flattened to 2D
    w = nc.dram_tensor("w", [nh * 4 * 128, D], in_dt, kind="ExternalInput").ap()
    y = nc.dram_tensor("y", [BS, D], f32, kind="ExternalOutput").ap()

    with TileContext(nc) as tc:
        with (
            tc.tile_pool(name="const", bufs=1) as cpool,
            tc.tile_pool(name="yout", bufs=1) as ypool,
            tc.tile_pool(name="psy", bufs=1, space="PSUM") as psy,
        ):
            # load inputs: 2 DMAs per (tensor, half) — chunk pairs j01 / j23 —
            # interleaved x/w so the first matmuls can start ~2us in
            xts, wts = [], []
            for h in range(nh):
                xth = cpool.tile([128, 4 * BS], in_dt, tag=f"xt{h}")
                xts.append(xth)
                wh = cpool.tile([128, 4 * D], in_dt, tag=f"w{h}")
                wts.append(wh)
            if nh == 1:
                # per-chunk DMAs (8 x ~0.25MB, interleaved x/w): finest
                # pipelining of arrivals against conversions and matmuls
                for j in range(4):
                    nc.sync.dma_start(
                        xts[0][:, j * BS : (j + 1) * BS], xt[j * 128 : (j + 1) * 128, :]
                    )
                    nc.sync.dma_start(
                        wts[0][:, j * D : (j + 1) * D], w[j * 128 : (j + 1) * 128, :]
                    )
            else:
                for h in range(nh):
                    for jp in range(2):  # chunk pair: j in {2*jp, 2*jp+1}
                        r0 = h * 512 + jp * 256
                        src = xt[r0 : r0 + 256, :].rearrange(
                            "(j p) b -> p j b", p=128
                        )
                        dst = xts[h][:, jp * 2 * BS : (jp + 1) * 2 * BS].rearrange(
                            "p (j b) -> p j b", j=2
                        )
                        nc.sync.dma_start(dst, src)
                        wsrc = w[r0 : r0 + 256, :].rearrange(
                            "(j p) k -> p j k", p=128
                        )
                        wdst = wts[h][:, jp * 2 * D : (jp + 1) * 2 * D].rearrange(
                            "p (j k) -> p j k", j=2
                        )
                        nc.sync.dma_start(wdst, wsrc)

            if precision == "f32r3":
                # exact hi/lo split on device: copy-to-f32r rounds, subtract
                # leaves the residual (fits the f32r mantissa) -> 3 f32r
                # matmul terms give fp32-grade accuracy at 1 cyc/row
                xh_t = cpool.tile([128, 4 * BS], f32r, tag="xh")
                xl_t = cpool.tile([128, 4 * BS], f32r, tag="xl")
                wh_t = cpool.tile([128, 4 * D], f32r, tag="wh")
                wl_t = cpool.tile([128, 4 * D], f32r, tag="wl")
                for j in range(4):
                    xsl = slice(j * BS, (j + 1) * BS)
                    nc.vector.tensor_copy(xh_t[:, xsl], xts[0][:, xsl])
                    nc.vector.tensor_sub(
                        xl_t[:, xsl], xts[0][:, xsl], xh_t[:, xsl].bitcast(f32)
                    )
                    wsl = slice(j * D, (j + 1) * D)
                    nc.scalar.copy(wh_t[:, wsl], wts[0][:, wsl])
                    nc.vector.tensor_sub(
                        wl_t[:, wsl], wts[0][:, wsl], wh_t[:, wsl].bitcast(f32)
                    )
                xts_mm, wts_mm = [xh_t, xl_t], [wh_t, wl_t]
                terms = [(0, 0), (1, 0), (0, 1)]
            else:
                xts_mm, wts_mm = xts, wts
                # terms: (x half, w half); hi*hi, lo*hi, hi*lo
                terms = [(0, 0), (1, 0), (0, 1)] if split else [(0, 0)]
            ypss = []
            for g in range(NT):
                yps = psy.tile([128, D], f32, tag=f"yps{g}")
                ypss.append(yps)
            def mm(g, t, xh, wh, j, start, stop):
                nc.tensor.matmul(
                    ypss[g][:],
                    lhsT=xts_mm[xh][
                        0 : LCH[j], j * BS + g * 128 : j * BS + (g + 1) * 128
                    ],
                    rhs=wts_mm[wh][0 : LCH[j], j * D : (j + 1) * D],
                    start=start, stop=stop,
                )

            # term 0 for all groups first (only needs the first-half inputs),
            # then per-group remaining terms back-to-back: group completions
            # stagger ~every 8 matmuls and the out-path pipelines behind PE
            for g in range(NT):
                for j in range(4):
                    mm(g, 0, *terms[0], j, start=(j == 0),
                       stop=(len(terms) == 1 and j == 3))
            for g in range(NT):
                for t in range(1, len(terms)):
                    for j in range(4):
                        mm(g, t, *terms[t], j, start=False,
                           stop=(t == len(terms) - 1 and j == 3))
                ys = ypool.tile([128, D], f32, tag=f"ys{g}")
                if g % 2 == 0:
                    nc.vector.tensor_copy(ys[:], ypss[g][:])
                else:
                    nc.scalar.copy(ys[:], ypss[g][:])
                nc.sync.dma_start(y[g * 128 : (g + 1) * 128, :], ys[:])
    nc.compile()
    return nc


def _prep_inputs(input_state, angles, precision):
    """Host marshalling: weight build, transpose, chunk-pad, bf16 split."""
    Mt = _host_weight(angles)  # [l, k] float64
    x64 = np.asarray(input_state, dtype=np.float64)
    if precision == "bf16x3":
        w32 = Mt.astype(np.float32)
        wa, wb = _bf16_pair(w32)
        wcat = np.concatenate([_chunk_pad(wa), _chunk_pad(wb)], axis=0)
        w_host = wcat.reshape(8 * 128, D)
        x32 = x64.astype(np.float32)
        xa, xb = _bf16_pair(x32)
        xt_full = [
            np.concatenate(
                [_chunk_pad(np.ascontiguousarray(h.T)) for h in (xa, xb)], axis=0
            ).reshape(8 * 128, BATCH)
        ][0]
    else:
        w_host = _chunk_pad(Mt.astype(np.float32)).reshape(4 * 128, D)
        xt_full = _chunk_pad(
            np.ascontiguousarray(x64.astype(np.float32).T)
        ).reshape(4 * 128, BATCH)
    in_maps = [
        {"xt": np.ascontiguousarray(xt_full[:, c * BS : (c + 1) * BS]), "w": w_host}
        for c in range(NCORES)
    ]
    return in_maps


def run_device(input_state, angles, trace=False, precision=DEFAULT_PRECISION,
               **trace_kw):
    """Shard, run on 8 cores, gather. Returns (out, BassKernelResults)."""
    if precision not in _NC:
        _NC[precision] = _build_module(precision)
    from concourse import bass_utils

    in_maps = _prep_inputs(input_state, angles, precision)
    res = bass_utils.run_bass_kernel_spmd(
        _NC[precision], in_maps, core_ids=list(range(NCORES)), trace=trace,
        **trace_kw
    )
    out = np.concatenate([res.results[c]["y"] for c in range(NCORES)], axis=0)
    return np.ascontiguousarray(out.astype(np.float32)), res


def kernel(input_state, angles, U=None, **_ignored) -> np.ndarray:
    out, _ = run_device(input_state, angles, trace=False)
    return out



# revision 2
# speedup vs baseline: 1.9417x; 1.9417x over previous
"""Trainium2 kernel for nn_Dense_RBS_state_vector (v7).

Math: each RBS gate on the Hamming-weight-2 basis is the second exterior
power of a 32x32 Givens rotation; the 62-gate scan collapses to one dense
[496,496] matrix  W = Lambda^2(R)^T, so the whole reference is one matmul
y = x @ W. R (and hence W) is computed on the host in float64 from the
runtime angles.

Structure exploited on device: R is banded (R[i,j] = 0 for j > i+2 exactly),
so with input features sorted by pair-max (b) and output pairs sorted by
d-descending, each 128-row contraction chunk j only feeds a prefix of
n_j = C(B_j+3, 2) output columns — 1358 of 4*496 chunk-column products
survive (32% of the matmul work and W bytes skipped, exactly).

Device kernel (per core, data-parallel over 8 cores, all bf16):
  - 6 input DMAs [W-chunk block | x^T chunk] sized/ordered so the serial
    DMA device and the per-issue HWDGE/DGE pipeline stay saturated; two
    issues ride the Pool/SWDGE path to widen the issue chain.
  - 3 PE warmup matmuls + piece-gated waves exploit the cost-model p-state
    ramp (visits after ~3.7us run at 2.4 GHz).
  - Matmul waves per chunk (N = prefix width), accumulating into two
    [128,1024] f32 PSUM pair-tiles (groups bank-aligned at col 0/512);
    the last chunk's waves interleave per batch-group to stagger PSUM
    completions.
  - One full-pair PSUM->SBUF copy per pair (ACT: groups 0-1, DVE: 2-3),
    then two pair DMAs SBUF->DRAM on the SP queue.

Accuracy: bf16 x/W/y gives rel err ~2.9e-3 (gate: 2e-2).
"""

import numpy as np

N_QUBITS = 32
D = 496
BATCH = 4096
NCORES = 8
BS = BATCH // NCORES  # 512
GATES = [(i, i + 1) for i in range(N_QUBITS - 1)] * 2
LSPLIT = [0, 128, 256, 384, 496]
ROWS = [128, 128, 128, 112]
C0SPLIT = 248
C1SPLIT = 203

DEFAULT_PRECISION = "bf16s"
_NC = {}


def _orderings():
    pairs = [(a, b) for a in range(N_QUBITS) for b in range(a + 1, N_QUBITS)]
    lperm = sorted(range(D), key=lambda k: (pairs[k][1], pairs[k][0]))
    kperm = sorted(range(D), key=lambda k: (-pairs[k][1], pairs[k][0]))
    bmin = [min(pairs[lperm[i]][1] for i in range(LSPLIT[j], LSPLIT[j + 1]))
            for j in range(4)]
    nj = [sum(1 for k in kperm if pairs[k][1] >= bmin[j] - 2) for j in range(4)]
    return pairs, lperm, kperm, nj


def _host_weight_blocks(angles):
    pairs, lperm, kperm, nj = _orderings()
    R = np.eye(N_QUBITS, dtype=np.float64)
    for (i, j), th in zip(GATES, np.asarray(angles, dtype=np.float64)):
        c, s = np.cos(th), np.sin(th)
        Ri, Rj = R[i].copy(), R[j].copy()
        R[i] = c * Ri + s * Rj
        R[j] = -s * Ri + c * Rj
    A = np.asarray(pairs)
    a_, b_ = A[:, 0], A[:, 1]
    M = (R[np.ix_(a_, a_)] * R[np.ix_(b_, b_)]
         - R[np.ix_(a_, b_)] * R[np.ix_(b_, a_)])  # M[k, l]
    W = M.T[np.ix_(lperm, kperm)]  # W_used[l, k], permuted
    blocks = []
    for j in range(4):
        rows = W[LSPLIT[j] : LSPLIT[j + 1], : nj[j]]
        blk = np.zeros((ROWS[j], nj[j]), dtype=np.float64)
        blk[: rows.shape[0]] = rows
        blocks.append(blk)
    return blocks, lperm, kperm, nj


def _build_module(precision=DEFAULT_PRECISION):
    import concourse.bacc as bacc
    import concourse.mybir as mybir
    from concourse.tile import TileContext

    _, _, _, nj = _orderings()
    bf16 = mybir.dt.bfloat16
    f32 = mybir.dt.float32
    nc = bacc.Bacc("TRN2", target_bir_lowering=False, debug=False)

    piece_shapes = {
        "3": (ROWS[3], nj[3] + BS),
        "2": (ROWS[2], nj[2] + BS),
        "1a": (ROWS[1], C1SPLIT + BS),
        "1b": (ROWS[1], nj[1] - C1SPLIT),
        "0a": (ROWS[0], C0SPLIT + BS),
        "0b": (ROWS[0], nj[0] - C0SPLIT),
    }
    drt = {k: nc.dram_tensor(f"in{k}", list(s), bf16, kind="ExternalInput").ap()
           for k, s in piece_shapes.items()}
    y = nc.dram_tensor("y", [BS, D], bf16, kind="ExternalOutput").ap()

    in_assign = (("sync", "3"), ("gpsimd", "2"), ("sync", "1a"),
                 ("gpsimd", "1b"), ("sync", "0a"), ("sync", "0b"))
    eng = {"sync": nc.sync, "scalar": nc.scalar, "vector": nc.vector,
           "gpsimd": nc.gpsimd}

    with TileContext(nc) as tc:
        with (
            tc.tile_pool(name="const", bufs=1) as cpool,
            tc.tile_pool(name="yout", bufs=1) as ypool,
            tc.tile_pool(name="psy", bufs=1, space="PSUM") as psy,
        ):
            ptiles = {}
            for e_name, key in in_assign:
                t = cpool.tile(list(piece_shapes[key]), bf16,
                               tag=f"p{key}", name=f"p{key}")
                ptiles[key] = t
                eng[e_name].dma_start(t[:], drt[key])

            # PE warmup: sets pe_busy_start early so real waves visit warm
            scratch = cpool.tile([128, 128], bf16, tag="scr", name="scr")
            nc.vector.memset(scratch[:], 1.0)
            wps = psy.tile([128, 128], f32, tag="wps", name="wps")
            for _ in range(3):
                nc.tensor.matmul(wps[:], lhsT=scratch[:, 0:128],
                                 rhs=scratch[:], start=True, stop=True)
            # preload ACT tables off the critical path
            asc = cpool.tile([1, 8], f32, tag="asc", name="asc")
            nc.vector.memset(asc[:], 0.0)
            ad = cpool.tile([1, 8], bf16, tag="ad", name="ad")
            nc.scalar.copy(ad[:], asc[:])

            pairs = [psy.tile([128, 1024], f32, tag=f"pp{i}", name=f"pp{i}")
                     for i in range(2)]

            def ps(g, lo, hi):
                pi, s = divmod(g, 2)
                return pairs[pi][:, s * 512 + lo : s * 512 + hi]

            for wi, key in enumerate(("3", "2")):
                j = int(key)
                t = ptiles[key]
                for g in range(4):
                    nc.tensor.matmul(
                        ps(g, 0, nj[j]),
                        lhsT=t[0 : ROWS[j], nj[j] + g * 128 : nj[j] + (g + 1) * 128],
                        rhs=t[0 : ROWS[j], 0 : nj[j]],
                        start=(wi == 0), stop=False,
                    )
            t1a, t1b = ptiles["1a"], ptiles["1b"]
            t0a, t0b = ptiles["0a"], ptiles["0b"]
            for g in range(4):
                nc.tensor.matmul(
                    ps(g, 0, C1SPLIT),
                    lhsT=t1a[0 : ROWS[1], C1SPLIT + g * 128 : C1SPLIT + (g + 1) * 128],
                    rhs=t1a[0 : ROWS[1], 0:C1SPLIT],
                    start=False, stop=False,
                )

            for g in range(4):
                nc.tensor.matmul(
                    ps(g, C1SPLIT, nj[1]),
                    lhsT=t1a[0 : ROWS[1],
                             C1SPLIT + g * 128 : C1SPLIT + (g + 1) * 128],
                    rhs=t1b[0 : ROWS[1], 0 : nj[1] - C1SPLIT],
                    start=False, stop=False,
                )
            # tail: per-group [c0a-g, c0b-g] to stagger PSUM completions
            for g in range(4):
                nc.tensor.matmul(
                    ps(g, 0, C0SPLIT),
                    lhsT=t0a[0 : ROWS[0],
                             C0SPLIT + g * 128 : C0SPLIT + (g + 1) * 128],
                    rhs=t0a[0 : ROWS[0], 0:C0SPLIT],
                    start=False, stop=False,
                )
                nc.tensor.matmul(
                    ps(g, C0SPLIT, nj[0]),
                    lhsT=t0a[0 : ROWS[0],
                             C0SPLIT + g * 128 : C0SPLIT + (g + 1) * 128],
                    rhs=t0b[0 : ROWS[0], 0 : nj[0] - C0SPLIT],
                    start=False, stop=True,
                )

            ysp = [ypool.tile([128, 2 * D], bf16, tag=f"ysp{i}", name=f"ysp{i}")
                   for i in range(2)]
            for pi, e_name in enumerate(("scalar", "vector")):
                src = pairs[pi][:].rearrange("p (g k) -> p g k", g=2)[:, :, 0:D]
                dst = ysp[pi][:].rearrange("p (g k) -> p g k", g=2)
                if e_name == "scalar":
                    eng[e_name].copy(dst, src)
                else:
                    eng[e_name].tensor_copy(dst, src)

            for pi in range(2):
                nc.sync.dma_start(
                    y[pi * 256 : (pi + 1) * 256, :].rearrange(
                        "(g b) k -> b g k", g=2),
                    ysp[pi][:].rearrange("b (g k) -> b g k", g=2),
                )
    nc.compile()
    return nc


def _prep_inputs(input_state, angles):
    import ml_dtypes

    blocks, lperm, kperm, nj = _host_weight_blocks(angles)
    x = np.asarray(input_state, dtype=np.float64)
    xp = x[:, lperm]
    in_maps = []
    for c in range(NCORES):
        xc = xp[c * BS : (c + 1) * BS]
        m = {}
        for j, key in ((3, "3"), (2, "2")):
            r = ROWS[j]
            piece = np.zeros((r, nj[j] + BS), dtype=np.float64)
            piece[:, : nj[j]] = blocks[j]
            piece[: LSPLIT[j + 1] - LSPLIT[j], nj[j] :] = (
                xc[:, LSPLIT[j] : LSPLIT[j + 1]].T)
            m[f"in{key}"] = piece.astype(ml_dtypes.bfloat16)
        for j, split, ka, kb in ((1, C1SPLIT, "1a", "1b"),
                                 (0, C0SPLIT, "0a", "0b")):
            r = ROWS[j]
            pa = np.zeros((r, split + BS), dtype=np.float64)
            pa[:, :split] = blocks[j][:, :split]
            pa[: LSPLIT[j + 1] - LSPLIT[j], split:] = (
                xc[:, LSPLIT[j] : LSPLIT[j + 1]].T)
            m[f"in{ka}"] = pa.astype(ml_dtypes.bfloat16)
            m[f"in{kb}"] = np.ascontiguousarray(
                blocks[j][:, split:]).astype(ml_dtypes.bfloat16)
        in_maps.append(m)
    return in_maps, kperm


def run_device(input_state, angles, trace=False, precision=DEFAULT_PRECISION,
               **trace_kw):
    """Shard, run on 8 cores, gather. Returns (out, BassKernelResults)."""
    if precision not in _NC:
        _NC[precision] = _build_module(precision)
    from concourse import bass_utils

    in_maps, kperm = _prep_inputs(input_state, angles)
    res = bass_utils.run_bass_kernel_spmd(
        _NC[precision], in_maps, core_ids=list(range(NCORES)), trace=trace,
        **trace_kw
    )
    out = np.concatenate(
        [np.asarray(res.results[c]["y"], dtype=np.float32)
         for c in range(NCORES)], axis=0)
    inv = np.argsort(kperm)
    out = np.ascontiguousarray(out[:, inv])
    return out, res


def kernel(input_state, angles, U=None, **_ignored) -> np.ndarray:
    out, _ = run_device(input_state, angles, trace=False)
    return out


# revision 3
# speedup vs baseline: 1.9930x; 1.0264x over previous
"""Trainium2 kernel for nn_Dense_RBS_state_vector (v7).

Math: each RBS gate on the Hamming-weight-2 basis is the second exterior
power of a 32x32 Givens rotation; the 62-gate scan collapses to one dense
[496,496] matrix  W = Lambda^2(R)^T, so the whole reference is one matmul
y = x @ W. R (and hence W) is computed on the host in float64 from the
runtime angles.

Structure exploited on device: R is banded (R[i,j] = 0 for j > i+2 exactly),
so with input features sorted by pair-max (b) and output pairs sorted by
d-descending, each 128-row contraction chunk j only feeds a prefix of
n_j = C(B_j+3, 2) output columns — 1358 of 4*496 chunk-column products
survive (32% of the matmul work and W bytes skipped, exactly).

Device kernel (per core, data-parallel over 8 cores, all bf16):
  - 6 input DMAs [W-chunk block | x^T chunk] sized/ordered so the serial
    DMA device and the per-issue HWDGE/DGE pipeline stay saturated; two
    issues ride the Pool/SWDGE path to widen the issue chain.
  - 3 PE warmup matmuls + piece-gated waves exploit the cost-model p-state
    ramp (visits after ~3.7us run at 2.4 GHz).
  - Matmul waves per chunk (N = prefix width), accumulating into two
    [128,1024] f32 PSUM pair-tiles (groups bank-aligned at col 0/512);
    the last chunk's waves interleave per batch-group to stagger PSUM
    completions.
  - One full-pair PSUM->SBUF copy per pair (ACT: groups 0-1, DVE: 2-3),
    then two pair DMAs SBUF->DRAM on the SP queue.

Accuracy: bf16 x/W/y gives rel err ~2.9e-3 (gate: 2e-2).
"""

import numpy as np

N_QUBITS = 32
D = 496
BATCH = 4096
NCORES = 8
BS = BATCH // NCORES  # 512
GATES = [(i, i + 1) for i in range(N_QUBITS - 1)] * 2
LSPLIT = [0, 128, 256, 384, 496]
ROWS = [128, 128, 128, 112]
C0SPLIT = 216
C1SPLIT = 250

DEFAULT_PRECISION = "bf16s"
_NC = {}


def _orderings():
    pairs = [(a, b) for a in range(N_QUBITS) for b in range(a + 1, N_QUBITS)]
    lperm = sorted(range(D), key=lambda k: (pairs[k][1], pairs[k][0]))
    kperm = sorted(range(D), key=lambda k: (-pairs[k][1], pairs[k][0]))
    bmin = [min(pairs[lperm[i]][1] for i in range(LSPLIT[j], LSPLIT[j + 1]))
            for j in range(4)]
    nj = [sum(1 for k in kperm if pairs[k][1] >= bmin[j] - 2) for j in range(4)]
    return pairs, lperm, kperm, nj


def _host_weight_blocks(angles):
    pairs, lperm, kperm, nj = _orderings()
    R = np.eye(N_QUBITS, dtype=np.float64)
    for (i, j), th in zip(GATES, np.asarray(angles, dtype=np.float64)):
        c, s = np.cos(th), np.sin(th)
        Ri, Rj = R[i].copy(), R[j].copy()
        R[i] = c * Ri + s * Rj
        R[j] = -s * Ri + c * Rj
    A = np.asarray(pairs)
    a_, b_ = A[:, 0], A[:, 1]
    M = (R[np.ix_(a_, a_)] * R[np.ix_(b_, b_)]
         - R[np.ix_(a_, b_)] * R[np.ix_(b_, a_)])  # M[k, l]
    W = M.T[np.ix_(lperm, kperm)]  # W_used[l, k], permuted
    blocks = []
    for j in range(4):
        rows = W[LSPLIT[j] : LSPLIT[j + 1], : nj[j]]
        blk = np.zeros((ROWS[j], nj[j]), dtype=np.float64)
        blk[: rows.shape[0]] = rows
        blocks.append(blk)
    return blocks, lperm, kperm, nj


def _build_module(precision=DEFAULT_PRECISION):
    import concourse.bacc as bacc
    import concourse.mybir as mybir
    from concourse.tile import TileContext

    _, _, _, nj = _orderings()
    bf16 = mybir.dt.bfloat16
    f32 = mybir.dt.float32
    nc = bacc.Bacc("TRN2", target_bir_lowering=False, debug=False)

    piece_shapes = {
        "3": (ROWS[3], nj[3] + BS),
        "2": (ROWS[2], nj[2] + BS),
        "1a": (ROWS[1], C1SPLIT + BS),
        "1b": (ROWS[1], nj[1] - C1SPLIT),
        "0a": (ROWS[0], C0SPLIT + BS),
        "0b": (ROWS[0], nj[0] - C0SPLIT),
    }
    drt = {k: nc.dram_tensor(f"in{k}", list(s), bf16, kind="ExternalInput").ap()
           for k, s in piece_shapes.items()}
    y = nc.dram_tensor("y", [BS, D], bf16, kind="ExternalOutput").ap()

    in_assign = (("sync", "3"), ("gpsimd", "2"), ("sync", "1a"),
                 ("gpsimd", "0a"), ("sync", "1b"), ("sync", "0b"))
    eng = {"sync": nc.sync, "scalar": nc.scalar, "vector": nc.vector,
           "gpsimd": nc.gpsimd}

    with TileContext(nc) as tc:
        with (
            tc.tile_pool(name="const", bufs=1) as cpool,
            tc.tile_pool(name="yout", bufs=1) as ypool,
            tc.tile_pool(name="psy", bufs=1, space="PSUM") as psy,
        ):
            ptiles = {}
            for e_name, key in in_assign:
                t = cpool.tile(list(piece_shapes[key]), bf16,
                               tag=f"p{key}", name=f"p{key}")
                ptiles[key] = t
                eng[e_name].dma_start(t[:], drt[key])

            # PE warmup: sets pe_busy_start early so real waves visit warm
            scratch = cpool.tile([128, 128], bf16, tag="scr", name="scr")
            nc.vector.memset(scratch[:], 1.0)
            wps = psy.tile([128, 128], f32, tag="wps", name="wps")
            for _ in range(3):
                nc.tensor.matmul(wps[:], lhsT=scratch[:, 0:128],
                                 rhs=scratch[:], start=True, stop=True)
            # preload ACT tables off the critical path
            asc = cpool.tile([1, 8], f32, tag="asc", name="asc")
            nc.vector.memset(asc[:], 0.0)
            ad = cpool.tile([1, 8], bf16, tag="ad", name="ad")
            nc.scalar.copy(ad[:], asc[:])

            pairs = [psy.tile([128, 1024], f32, tag=f"pp{i}", name=f"pp{i}")
                     for i in range(2)]

            def ps(g, lo, hi):
                pi, s = divmod(g, 2)
                return pairs[pi][:, s * 512 + lo : s * 512 + hi]

            for wi, key in enumerate(("3", "2")):
                j = int(key)
                t = ptiles[key]
                for g in range(4):
                    nc.tensor.matmul(
                        ps(g, 0, nj[j]),
                        lhsT=t[0 : ROWS[j], nj[j] + g * 128 : nj[j] + (g + 1) * 128],
                        rhs=t[0 : ROWS[j], 0 : nj[j]],
                        start=(wi == 0), stop=False,
                    )
            t1a, t1b = ptiles["1a"], ptiles["1b"]
            t0a, t0b = ptiles["0a"], ptiles["0b"]
            for g in range(4):
                nc.tensor.matmul(
                    ps(g, 0, C1SPLIT),
                    lhsT=t1a[0 : ROWS[1], C1SPLIT + g * 128 : C1SPLIT + (g + 1) * 128],
                    rhs=t1a[0 : ROWS[1], 0:C1SPLIT],
                    start=False, stop=False,
                )

            for g in range(4):
                nc.tensor.matmul(
                    ps(g, C1SPLIT, nj[1]),
                    lhsT=t1a[0 : ROWS[1],
                             C1SPLIT + g * 128 : C1SPLIT + (g + 1) * 128],
                    rhs=t1b[0 : ROWS[1], 0 : nj[1] - C1SPLIT],
                    start=False, stop=False,
                )
            # tail: per-group [c0a-g, c0b-g] to stagger PSUM completions
            for g in range(4):
                nc.tensor.matmul(
                    ps(g, 0, C0SPLIT),
                    lhsT=t0a[0 : ROWS[0],
                             C0SPLIT + g * 128 : C0SPLIT + (g + 1) * 128],
                    rhs=t0a[0 : ROWS[0], 0:C0SPLIT],
                    start=False, stop=False,
                )
                nc.tensor.matmul(
                    ps(g, C0SPLIT, nj[0]),
                    lhsT=t0a[0 : ROWS[0],
                             C0SPLIT + g * 128 : C0SPLIT + (g + 1) * 128],
                    rhs=t0b[0 : ROWS[0], 0 : nj[0] - C0SPLIT],
                    start=False, stop=True,
                )

            ysp = [ypool.tile([128, 2 * D], bf16, tag=f"ysp{i}", name=f"ysp{i}")
                   for i in range(2)]
            for pi, e_name in enumerate(("scalar", "vector")):
                src = pairs[pi][:].rearrange("p (g k) -> p g k", g=2)[:, :, 0:D]
                dst = ysp[pi][:].rearrange("p (g k) -> p g k", g=2)
                if e_name == "scalar":
                    eng[e_name].copy(dst, src)
                else:
                    eng[e_name].tensor_copy(dst, src)

            for pi in range(2):
                nc.sync.dma_start(
                    y[pi * 256 : (pi + 1) * 256, :].rearrange(
                        "(g b) k -> b g k", g=2),
                    ysp[pi][:].rearrange("b (g k) -> b g k", g=2),
                )
    nc.compile()
    return nc


def _prep_inputs(input_state, angles):
    import ml_dtypes

    blocks, lperm, kperm, nj = _host_weight_blocks(angles)
    x = np.asarray(input_state, dtype=np.float64)
    xp = x[:, lperm]
    in_maps = []
    for c in range(NCORES):
        xc = xp[c * BS : (c + 1) * BS]
        m = {}
        for j, key in ((3, "3"), (2, "2")):
            r = ROWS[j]
            piece = np.zeros((r, nj[j] + BS), dtype=np.float64)
            piece[:, : nj[j]] = blocks[j]
            piece[: LSPLIT[j + 1] - LSPLIT[j], nj[j] :] = (
                xc[:, LSPLIT[j] : LSPLIT[j + 1]].T)
            m[f"in{key}"] = piece.astype(ml_dtypes.bfloat16)
        for j, split, ka, kb in ((1, C1SPLIT, "1a", "1b"),
                                 (0, C0SPLIT, "0a", "0b")):
            r = ROWS[j]
            pa = np.zeros((r, split + BS), dtype=np.float64)
            pa[:, :split] = blocks[j][:, :split]
            pa[: LSPLIT[j + 1] - LSPLIT[j], split:] = (
                xc[:, LSPLIT[j] : LSPLIT[j + 1]].T)
            m[f"in{ka}"] = pa.astype(ml_dtypes.bfloat16)
            m[f"in{kb}"] = np.ascontiguousarray(
                blocks[j][:, split:]).astype(ml_dtypes.bfloat16)
        in_maps.append(m)
    return in_maps, kperm


def run_device(input_state, angles, trace=False, precision=DEFAULT_PRECISION,
               **trace_kw):
    """Shard, run on 8 cores, gather. Returns (out, BassKernelResults)."""
    if precision not in _NC:
        _NC[precision] = _build_module(precision)
    from concourse import bass_utils

    in_maps, kperm = _prep_inputs(input_state, angles)
    res = bass_utils.run_bass_kernel_spmd(
        _NC[precision], in_maps, core_ids=list(range(NCORES)), trace=trace,
        **trace_kw
    )
    out = np.concatenate(
        [np.asarray(res.results[c]["y"], dtype=np.float32)
         for c in range(NCORES)], axis=0)
    inv = np.argsort(kperm)
    out = np.ascontiguousarray(out[:, inv])
    return out, res


def kernel(input_state, angles, U=None, **_ignored) -> np.ndarray:
    out, _ = run_device(input_state, angles, trace=False)
    return out


# revision 4
# speedup vs baseline: 2.0317x; 1.0194x over previous
"""Trainium2 kernel for nn_Dense_RBS_state_vector (v9).

Math: each RBS gate on the Hamming-weight-2 basis is the second exterior
power of a 32x32 Givens rotation; the 62-gate scan collapses to one dense
[496,496] matrix W = Lambda^2(R)^T, so the whole reference is one matmul
y = x @ W. R (and hence W) is computed on the host in float64 from the
runtime angles.

Structure exploited on device: R is banded (R[i,j] = 0 for j > i+2 exactly),
so with input features sorted by pair-max (b) and output pairs sorted by
d-descending, each 128-row contraction chunk j only feeds a prefix of
n_j = C(B_j+3, 2) output columns — (496, 405, 286, 171) widths: 32% of
matmul cycles and W bytes skipped, exactly (dropped blocks are identically
zero in the reference too).

Device kernel (per core, data-parallel over 8 cores, all bf16):
  - 7 input DMAs sized/ordered to saturate the serial DMA device and the
    per-issue HWDGE/DGE pipeline; three ride the Pool/SWDGE path to widen
    the issue chain. Chunk 0's x block is split by batch-group so groups
    0/1 stop depending on the last transfer's +900ns completion sem.
  - 3 PE warmup matmuls + piece-gated waves exploit the cost-model p-state
    ramp (visits after ~3.7us are charged at 2.4 GHz).
  - Matmul waves per chunk (N = prefix width, small chunks first),
    accumulating into two [128,1024] f32 PSUM pair-tiles (groups
    bank-aligned at col 0/512); the tail interleaves so groups 0/1 finish
    first (the output chain is anchored on their drain).
  - One full-pair strided PSUM->SBUF copy per pair (ACT: groups 0-1,
    DVE: 2-3), then two pair DMAs SBUF->DRAM on the SP queue.

Accuracy: bf16 x/W/y gives rel err ~2.9e-3 (harness gate: 2e-2).
"""

import numpy as np

N_QUBITS = 32
D = 496
BATCH = 4096
NCORES = 8
BS = BATCH // NCORES  # 512
GATES = [(i, i + 1) for i in range(N_QUBITS - 1)] * 2
LSPLIT = [0, 128, 256, 384, 496]
ROWS = [128, 128, 128, 112]
C1SPLIT = 340

DEFAULT_PRECISION = "bf16s"
_NC = {}


def _orderings():
    pairs = [(a, b) for a in range(N_QUBITS) for b in range(a + 1, N_QUBITS)]
    lperm = sorted(range(D), key=lambda k: (pairs[k][1], pairs[k][0]))
    kperm = sorted(range(D), key=lambda k: (-pairs[k][1], pairs[k][0]))
    bmin = [min(pairs[lperm[i]][1] for i in range(LSPLIT[j], LSPLIT[j + 1]))
            for j in range(4)]
    nj = [sum(1 for k in kperm if pairs[k][1] >= bmin[j] - 2) for j in range(4)]
    return pairs, lperm, kperm, nj


def _host_weight_blocks(angles):
    pairs, lperm, kperm, nj = _orderings()
    R = np.eye(N_QUBITS, dtype=np.float64)
    for (i, j), th in zip(GATES, np.asarray(angles, dtype=np.float64)):
        c, s = np.cos(th), np.sin(th)
        Ri, Rj = R[i].copy(), R[j].copy()
        R[i] = c * Ri + s * Rj
        R[j] = -s * Ri + c * Rj
    A = np.asarray(pairs)
    a_, b_ = A[:, 0], A[:, 1]
    M = (R[np.ix_(a_, a_)] * R[np.ix_(b_, b_)]
         - R[np.ix_(a_, b_)] * R[np.ix_(b_, a_)])  # M[k, l]
    W = M.T[np.ix_(lperm, kperm)]  # W_used[l, k], permuted
    blocks = []
    for j in range(4):
        rows = W[LSPLIT[j] : LSPLIT[j + 1], : nj[j]]
        blk = np.zeros((ROWS[j], nj[j]), dtype=np.float64)
        blk[: rows.shape[0]] = rows
        blocks.append(blk)
    return blocks, lperm, kperm, nj


def _build_module(precision=DEFAULT_PRECISION):
    import concourse.bacc as bacc
    import concourse.mybir as mybir
    from concourse.tile import TileContext

    _, _, _, nj = _orderings()
    bf16 = mybir.dt.bfloat16
    f32 = mybir.dt.float32
    nc = bacc.Bacc("TRN2", target_bir_lowering=False, debug=False)

    piece_shapes = {
        "3": (ROWS[3], nj[3] + BS),
        "2": (ROWS[2], nj[2] + BS),
        "1a": (ROWS[1], C1SPLIT + BS),
        "1b": (ROWS[1], nj[1] - C1SPLIT),
        "0w": (ROWS[0], nj[0]),
        "0x01": (ROWS[0], 256),
        "0x23": (ROWS[0], 256),
    }
    in_assign = (("sync", "3"), ("gpsimd", "2"), ("sync", "1a"),
                 ("gpsimd", "1b"), ("sync", "0w"), ("sync", "0x01"),
                 ("gpsimd", "0x23"))
    drt = {k: nc.dram_tensor(f"in{k}", list(s), bf16, kind="ExternalInput").ap()
           for k, s in piece_shapes.items()}
    y = nc.dram_tensor("y", [BS, D], bf16, kind="ExternalOutput").ap()
    eng = {"sync": nc.sync, "scalar": nc.scalar, "vector": nc.vector,
           "gpsimd": nc.gpsimd}

    with TileContext(nc) as tc:
        with (
            tc.tile_pool(name="const", bufs=1) as cpool,
            tc.tile_pool(name="yout", bufs=1) as ypool,
            tc.tile_pool(name="psy", bufs=1, space="PSUM") as psy,
        ):
            ptiles = {}
            for e_name, key in in_assign:
                t = cpool.tile(list(piece_shapes[key]), bf16,
                               tag=f"p{key}", name=f"p{key}")
                ptiles[key] = t
                eng[e_name].dma_start(t[:], drt[key])

            # PE warmup: establishes pe_busy_start early so real waves,
            # visited after their piece sems (>3.7us), are charged warm.
            scratch = cpool.tile([128, 128], bf16, tag="scr", name="scr")
            nc.vector.memset(scratch[:], 1.0)
            wps = psy.tile([128, 128], f32, tag="wps", name="wps")
            for _ in range(3):
                nc.tensor.matmul(wps[:], lhsT=scratch[:, 0:128],
                                 rhs=scratch[:], start=True, stop=True)
            # preload ACT tables off the critical path
            asc = cpool.tile([1, 8], f32, tag="asc", name="asc")
            nc.vector.memset(asc[:], 0.0)
            ad = cpool.tile([1, 8], bf16, tag="ad", name="ad")
            nc.scalar.copy(ad[:], asc[:])

            pairs = [psy.tile([128, 1024], f32, tag=f"pp{i}", name=f"pp{i}")
                     for i in range(2)]

            def ps(g, lo, hi):
                pi, s = divmod(g, 2)
                return pairs[pi][:, s * 512 + lo : s * 512 + hi]

            for wi, key in enumerate(("3", "2")):
                j = int(key)
                t = ptiles[key]
                for g in range(4):
                    nc.tensor.matmul(
                        ps(g, 0, nj[j]),
                        lhsT=t[0 : ROWS[j], nj[j] + g * 128 : nj[j] + (g + 1) * 128],
                        rhs=t[0 : ROWS[j], 0 : nj[j]],
                        start=(wi == 0), stop=False,
                    )
            t1a, t1b = ptiles["1a"], ptiles["1b"]
            for g in range(4):
                nc.tensor.matmul(
                    ps(g, 0, C1SPLIT),
                    lhsT=t1a[0 : ROWS[1], C1SPLIT + g * 128 : C1SPLIT + (g + 1) * 128],
                    rhs=t1a[0 : ROWS[1], 0:C1SPLIT],
                    start=False, stop=False,
                )
            # tail: groups 0/1 fully finish (c1b, c0), then groups 2/3
            t0w = ptiles["0w"]
            tx = {0: ptiles["0x01"], 1: ptiles["0x01"],
                  2: ptiles["0x23"], 3: ptiles["0x23"]}
            for gs in ((0, 1), (2, 3)):
                for g in gs:
                    nc.tensor.matmul(
                        ps(g, C1SPLIT, nj[1]),
                        lhsT=t1a[0 : ROWS[1],
                                 C1SPLIT + g * 128 : C1SPLIT + (g + 1) * 128],
                        rhs=t1b[0 : ROWS[1], 0 : nj[1] - C1SPLIT],
                        start=False, stop=False,
                    )
                for g in gs:
                    xo = (g % 2) * 128
                    nc.tensor.matmul(
                        ps(g, 0, nj[0]),
                        lhsT=tx[g][0 : ROWS[0], xo : xo + 128],
                        rhs=t0w[0 : ROWS[0], 0 : nj[0]],
                        start=False, stop=True,
                    )

            ysp = [ypool.tile([128, 2 * D], bf16, tag=f"ysp{i}", name=f"ysp{i}")
                   for i in range(2)]
            for pi, e_name in enumerate(("scalar", "vector")):
                src = pairs[pi][:].rearrange("p (g k) -> p g k", g=2)[:, :, 0:D]
                dst = ysp[pi][:].rearrange("p (g k) -> p g k", g=2)
                if e_name == "scalar":
                    eng[e_name].copy(dst, src)
                else:
                    eng[e_name].tensor_copy(dst, src)
            for pi in range(2):
                nc.sync.dma_start(
                    y[pi * 256 : (pi + 1) * 256, :].rearrange(
                        "(g b) k -> b g k", g=2),
                    ysp[pi][:].rearrange("b (g k) -> b g k", g=2),
                )
    nc.compile()
    return nc


def _prep_inputs(input_state, angles):
    import ml_dtypes

    blocks, lperm, kperm, nj = _host_weight_blocks(angles)
    x = np.asarray(input_state, dtype=np.float64)
    xp = x[:, lperm]
    in_maps = []
    for c in range(NCORES):
        xc = xp[c * BS : (c + 1) * BS]
        m = {}
        for j, key in ((3, "3"), (2, "2")):
            r = ROWS[j]
            piece = np.zeros((r, nj[j] + BS), dtype=np.float64)
            piece[:, : nj[j]] = blocks[j]
            piece[: LSPLIT[j + 1] - LSPLIT[j], nj[j] :] = (
                xc[:, LSPLIT[j] : LSPLIT[j + 1]].T)
            m[f"in{key}"] = piece.astype(ml_dtypes.bfloat16)
        r = ROWS[1]
        pa = np.zeros((r, C1SPLIT + BS), dtype=np.float64)
        pa[:, :C1SPLIT] = blocks[1][:, :C1SPLIT]
        pa[:r, C1SPLIT:] = xc[:, LSPLIT[1] : LSPLIT[2]].T
        m["in1a"] = pa.astype(ml_dtypes.bfloat16)
        m["in1b"] = np.ascontiguousarray(blocks[1][:, C1SPLIT:]).astype(
            ml_dtypes.bfloat16)
        m["in0w"] = np.ascontiguousarray(blocks[0]).astype(ml_dtypes.bfloat16)
        x0 = xc[:, LSPLIT[0] : LSPLIT[1]].T  # [128, 512]
        m["in0x01"] = np.ascontiguousarray(x0[:, 0:256]).astype(ml_dtypes.bfloat16)
        m["in0x23"] = np.ascontiguousarray(x0[:, 256:512]).astype(ml_dtypes.bfloat16)
        in_maps.append(m)
    return in_maps, kperm


def run_device(input_state, angles, trace=False, precision=DEFAULT_PRECISION,
               **trace_kw):
    """Shard, run on 8 cores, gather. Returns (out, BassKernelResults)."""
    if precision not in _NC:
        _NC[precision] = _build_module(precision)
    from concourse import bass_utils

    in_maps, kperm = _prep_inputs(input_state, angles)
    res = bass_utils.run_bass_kernel_spmd(
        _NC[precision], in_maps, core_ids=list(range(NCORES)), trace=trace,
        **trace_kw
    )
    out = np.concatenate(
        [np.asarray(res.results[c]["y"], dtype=np.float32)
         for c in range(NCORES)], axis=0)
    inv = np.argsort(kperm)
    out = np.ascontiguousarray(out[:, inv])
    return out, res


def kernel(input_state, angles, U=None, **_ignored) -> np.ndarray:
    out, _ = run_device(input_state, angles, trace=False)
    return out


# revision 8
# speedup vs baseline: 2.0404x; 1.0043x over previous
"""Trainium2 kernel for nn_Dense_RBS_state_vector (v9).

Math: each RBS gate on the Hamming-weight-2 basis is the second exterior
power of a 32x32 Givens rotation; the 62-gate scan collapses to one dense
[496,496] matrix W = Lambda^2(R)^T, so the whole reference is one matmul
y = x @ W. R (and hence W) is computed on the host in float64 from the
runtime angles.

Structure exploited on device: R is banded (R[i,j] = 0 for j > i+2 exactly),
so with input features sorted by pair-max (b) and output pairs sorted by
d-descending, each 128-row contraction chunk j only feeds a prefix of
n_j = C(B_j+3, 2) output columns — (496, 405, 286, 171) widths: 32% of
matmul cycles and W bytes skipped, exactly (dropped blocks are identically
zero in the reference too).

Device kernel (per core, data-parallel over 8 cores, all bf16):
  - 7 input DMAs sized/ordered to saturate the serial DMA device and the
    per-issue HWDGE/DGE pipeline; three ride the Pool/SWDGE path to widen
    the issue chain. Chunk 0's x block is split by batch-group so groups
    0/1 stop depending on the last transfer's +900ns completion sem.
  - 3 PE warmup matmuls + piece-gated waves exploit the cost-model p-state
    ramp (visits after ~3.7us are charged at 2.4 GHz).
  - Matmul waves per chunk (N = prefix width, small chunks first),
    accumulating into two [128,1024] f32 PSUM pair-tiles (groups
    bank-aligned at col 0/512); the tail interleaves so groups 0/1 finish
    first (the output chain is anchored on their drain).
  - One full-pair strided PSUM->SBUF copy per pair (ACT: groups 0-1,
    DVE: 2-3), then two pair DMAs SBUF->DRAM on the SP queue.

Accuracy: bf16 x/W/y gives rel err ~2.9e-3 (harness gate: 2e-2).
"""

import numpy as np

N_QUBITS = 32
D = 496
BATCH = 4096
NCORES = 8
BS = BATCH // NCORES  # 512
GATES = [(i, i + 1) for i in range(N_QUBITS - 1)] * 2
LSPLIT = [0, 128, 256, 384, 496]
ROWS = [128, 128, 128, 112]
C1SPLIT = 300

DEFAULT_PRECISION = "bf16s"
_NC = {}


def _orderings():
    pairs = [(a, b) for a in range(N_QUBITS) for b in range(a + 1, N_QUBITS)]
    lperm = sorted(range(D), key=lambda k: (pairs[k][1], pairs[k][0]))
    kperm = sorted(range(D), key=lambda k: (-pairs[k][1], pairs[k][0]))
    bmin = [min(pairs[lperm[i]][1] for i in range(LSPLIT[j], LSPLIT[j + 1]))
            for j in range(4)]
    nj = [sum(1 for k in kperm if pairs[k][1] >= bmin[j] - 2) for j in range(4)]
    return pairs, lperm, kperm, nj


def _host_weight_blocks(angles):
    pairs, lperm, kperm, nj = _orderings()
    R = np.eye(N_QUBITS, dtype=np.float64)
    for (i, j), th in zip(GATES, np.asarray(angles, dtype=np.float64)):
        c, s = np.cos(th), np.sin(th)
        Ri, Rj = R[i].copy(), R[j].copy()
        R[i] = c * Ri + s * Rj
        R[j] = -s * Ri + c * Rj
    A = np.asarray(pairs)
    a_, b_ = A[:, 0], A[:, 1]
    M = (R[np.ix_(a_, a_)] * R[np.ix_(b_, b_)]
         - R[np.ix_(a_, b_)] * R[np.ix_(b_, a_)])  # M[k, l]
    W = M.T[np.ix_(lperm, kperm)]  # W_used[l, k], permuted
    blocks = []
    for j in range(4):
        rows = W[LSPLIT[j] : LSPLIT[j + 1], : nj[j]]
        blk = np.zeros((ROWS[j], nj[j]), dtype=np.float64)
        blk[: rows.shape[0]] = rows
        blocks.append(blk)
    return blocks, lperm, kperm, nj


def _build_module(precision=DEFAULT_PRECISION):
    import concourse.bacc as bacc
    import concourse.mybir as mybir
    from concourse.tile import TileContext

    _, _, _, nj = _orderings()
    bf16 = mybir.dt.bfloat16
    f32 = mybir.dt.float32
    nc = bacc.Bacc("TRN2", target_bir_lowering=False, debug=False)

    rest = nj[1] - C1SPLIT  # w-chunk-1 tail columns, merged into the 0w piece
    piece_shapes = {
        "3": (ROWS[3], nj[3] + BS),
        "2": (ROWS[2], nj[2] + BS),
        "1a": (ROWS[1], C1SPLIT + BS),
        "0wb": (128, nj[0] + rest),
        "0x01": (ROWS[0], 256),
        "0x23": (ROWS[0], 256),
    }
    in_assign = (("sync", "3"), ("gpsimd", "2"), ("sync", "1a"),
                 ("sync", "0wb"), ("gpsimd", "0x01"), ("sync", "0x23"))
    drt = {k: nc.dram_tensor(f"in{k}", list(s), bf16, kind="ExternalInput").ap()
           for k, s in piece_shapes.items()}
    y = nc.dram_tensor("y", [BS, D], bf16, kind="ExternalOutput").ap()
    eng = {"sync": nc.sync, "scalar": nc.scalar, "vector": nc.vector,
           "gpsimd": nc.gpsimd}

    with TileContext(nc) as tc:
        with (
            tc.tile_pool(name="const", bufs=1) as cpool,
            tc.tile_pool(name="yout", bufs=1) as ypool,
            tc.tile_pool(name="psy", bufs=1, space="PSUM") as psy,
        ):
            ptiles = {}
            for e_name, key in in_assign:
                t = cpool.tile(list(piece_shapes[key]), bf16,
                               tag=f"p{key}", name=f"p{key}")
                ptiles[key] = t
                eng[e_name].dma_start(t[:], drt[key])

            # PE warmup: establishes pe_busy_start early so real waves,
            # visited after their piece sems (>3.7us), are charged warm.
            scratch = cpool.tile([128, 128], bf16, tag="scr", name="scr")
            nc.vector.memset(scratch[:], 1.0)
            wps = psy.tile([128, 128], f32, tag="wps", name="wps")
            for _ in range(3):
                nc.tensor.matmul(wps[:], lhsT=scratch[:, 0:128],
                                 rhs=scratch[:], start=True, stop=True)
            # preload ACT tables off the critical path
            asc = cpool.tile([1, 8], f32, tag="asc", name="asc")
            nc.vector.memset(asc[:], 0.0)
            ad = cpool.tile([1, 8], bf16, tag="ad", name="ad")
            nc.scalar.copy(ad[:], asc[:])

            pairs = [psy.tile([128, 1024], f32, tag=f"pp{i}", name=f"pp{i}")
                     for i in range(2)]

            def ps(g, lo, hi):
                pi, s = divmod(g, 2)
                return pairs[pi][:, s * 512 + lo : s * 512 + hi]

            for wi, key in enumerate(("3", "2")):
                j = int(key)
                t = ptiles[key]
                for g in range(4):
                    nc.tensor.matmul(
                        ps(g, 0, nj[j]),
                        lhsT=t[0 : ROWS[j], nj[j] + g * 128 : nj[j] + (g + 1) * 128],
                        rhs=t[0 : ROWS[j], 0 : nj[j]],
                        start=(wi == 0), stop=False,
                    )
            t1a = ptiles["1a"]
            for g in range(4):
                nc.tensor.matmul(
                    ps(g, 0, C1SPLIT),
                    lhsT=t1a[0 : ROWS[1], C1SPLIT + g * 128 : C1SPLIT + (g + 1) * 128],
                    rhs=t1a[0 : ROWS[1], 0:C1SPLIT],
                    start=False, stop=False,
                )
            # tail: groups 0/1 fully finish (c1b, c0), then groups 2/3
            t0w = ptiles["0wb"]
            tx = {0: ptiles["0x01"], 1: ptiles["0x01"],
                  2: ptiles["0x23"], 3: ptiles["0x23"]}
            for gs in ((0, 1), (2, 3)):
                for g in gs:
                    nc.tensor.matmul(
                        ps(g, C1SPLIT, nj[1]),
                        lhsT=t1a[0 : ROWS[1],
                                 C1SPLIT + g * 128 : C1SPLIT + (g + 1) * 128],
                        rhs=t0w[0 : ROWS[1], nj[0] : nj[0] + rest],
                        start=False, stop=False,
                    )
                for g in gs:
                    xo = (g % 2) * 128
                    nc.tensor.matmul(
                        ps(g, 0, nj[0]),
                        lhsT=tx[g][0 : ROWS[0], xo : xo + 128],
                        rhs=t0w[0 : ROWS[0], 0 : nj[0]],
                        start=False, stop=True,
                    )

            ysp = [ypool.tile([128, 2 * D], bf16, tag=f"ysp{i}", name=f"ysp{i}")
                   for i in range(2)]
            for pi, e_name in enumerate(("scalar", "vector")):
                src = pairs[pi][:].rearrange("p (g k) -> p g k", g=2)[:, :, 0:D]
                dst = ysp[pi][:].rearrange("p (g k) -> p g k", g=2)
                if e_name == "scalar":
                    eng[e_name].copy(dst, src)
                else:
                    eng[e_name].tensor_copy(dst, src)
            for pi in range(2):
                nc.sync.dma_start(
                    y[pi * 256 : (pi + 1) * 256, :].rearrange(
                        "(g b) k -> b g k", g=2),
                    ysp[pi][:].rearrange("b (g k) -> b g k", g=2),
                )
    nc.compile()
    return nc


def _prep_inputs(input_state, angles):
    import ml_dtypes

    blocks, lperm, kperm, nj = _host_weight_blocks(angles)
    x = np.asarray(input_state, dtype=np.float64)
    xp = x[:, lperm]
    in_maps = []
    for c in range(NCORES):
        xc = xp[c * BS : (c + 1) * BS]
        m = {}
        for j, key in ((3, "3"), (2, "2")):
            r = ROWS[j]
            piece = np.zeros((r, nj[j] + BS), dtype=np.float64)
            piece[:, : nj[j]] = blocks[j]
            piece[: LSPLIT[j + 1] - LSPLIT[j], nj[j] :] = (
                xc[:, LSPLIT[j] : LSPLIT[j + 1]].T)
            m[f"in{key}"] = piece.astype(ml_dtypes.bfloat16)
        r = ROWS[1]
        pa = np.zeros((r, C1SPLIT + BS), dtype=np.float64)
        pa[:, :C1SPLIT] = blocks[1][:, :C1SPLIT]
        pa[:r, C1SPLIT:] = xc[:, LSPLIT[1] : LSPLIT[2]].T
        m["in1a"] = pa.astype(ml_dtypes.bfloat16)
        m["in0wb"] = np.concatenate(
            [blocks[0], blocks[1][:, C1SPLIT:]], axis=1).astype(ml_dtypes.bfloat16)
        x0 = xc[:, LSPLIT[0] : LSPLIT[1]].T  # [128, 512]
        m["in0x01"] = np.ascontiguousarray(x0[:, 0:256]).astype(ml_dtypes.bfloat16)
        m["in0x23"] = np.ascontiguousarray(x0[:, 256:512]).astype(ml_dtypes.bfloat16)
        in_maps.append(m)
    return in_maps, kperm


def run_device(input_state, angles, trace=False, precision=DEFAULT_PRECISION,
               **trace_kw):
    """Shard, run on 8 cores, gather. Returns (out, BassKernelResults)."""
    if precision not in _NC:
        _NC[precision] = _build_module(precision)
    from concourse import bass_utils

    in_maps, kperm = _prep_inputs(input_state, angles)
    res = bass_utils.run_bass_kernel_spmd(
        _NC[precision], in_maps, core_ids=list(range(NCORES)), trace=trace,
        **trace_kw
    )
    out = np.concatenate(
        [np.asarray(res.results[c]["y"], dtype=np.float32)
         for c in range(NCORES)], axis=0)
    inv = np.argsort(kperm)
    out = np.ascontiguousarray(out[:, inv])
    return out, res


def kernel(input_state, angles, U=None, **_ignored) -> np.ndarray:
    out, _ = run_device(input_state, angles, trace=False)
    return out


# revision 12
# speedup vs baseline: 2.0584x; 1.0088x over previous
"""Trainium2 kernel for nn_Dense_RBS_state_vector (v9).

Math: each RBS gate on the Hamming-weight-2 basis is the second exterior
power of a 32x32 Givens rotation; the 62-gate scan collapses to one dense
[496,496] matrix W = Lambda^2(R)^T, so the whole reference is one matmul
y = x @ W. R (and hence W) is computed on the host in float64 from the
runtime angles.

Structure exploited on device: R is banded (R[i,j] = 0 for j > i+2 exactly),
so with input features sorted by pair-max (b) and output pairs sorted by
d-descending, each 128-row contraction chunk j only feeds a prefix of
n_j = C(B_j+3, 2) output columns — (496, 405, 286, 171) widths: 32% of
matmul cycles and W bytes skipped, exactly (dropped blocks are identically
zero in the reference too).

Device kernel (per core, data-parallel over 8 cores, all bf16):
  - 7 input DMAs sized/ordered to saturate the serial DMA device and the
    per-issue HWDGE/DGE pipeline; three ride the Pool/SWDGE path to widen
    the issue chain. Chunk 0's x block is split by batch-group so groups
    0/1 stop depending on the last transfer's +900ns completion sem.
  - 3 PE warmup matmuls + piece-gated waves exploit the cost-model p-state
    ramp (visits after ~3.7us are charged at 2.4 GHz).
  - Matmul waves per chunk (N = prefix width, small chunks first),
    accumulating into two [128,1024] f32 PSUM pair-tiles (groups
    bank-aligned at col 0/512); the tail interleaves so groups 0/1 finish
    first (the output chain is anchored on their drain).
  - One full-pair strided PSUM->SBUF copy per pair (ACT: groups 0-1,
    DVE: 2-3), then two pair DMAs SBUF->DRAM on the SP queue.

Accuracy: bf16 x/W/y gives rel err ~2.9e-3 (harness gate: 2e-2).
"""

import numpy as np

N_QUBITS = 32
D = 496
BATCH = 4096
NCORES = 8
BS = BATCH // NCORES  # 512
GATES = [(i, i + 1) for i in range(N_QUBITS - 1)] * 2
LSPLIT = [0, 128, 256, 384, 496]
ROWS = [128, 128, 128, 112]
C1SPLIT = 300

DEFAULT_PRECISION = "bf16s"
_NC = {}


def _orderings():
    pairs = [(a, b) for a in range(N_QUBITS) for b in range(a + 1, N_QUBITS)]
    lperm = sorted(range(D), key=lambda k: (pairs[k][1], pairs[k][0]))
    kperm = sorted(range(D), key=lambda k: (-pairs[k][1], pairs[k][0]))
    bmin = [min(pairs[lperm[i]][1] for i in range(LSPLIT[j], LSPLIT[j + 1]))
            for j in range(4)]
    nj = [sum(1 for k in kperm if pairs[k][1] >= bmin[j] - 2) for j in range(4)]
    return pairs, lperm, kperm, nj


def _host_weight_blocks(angles):
    pairs, lperm, kperm, nj = _orderings()
    R = np.eye(N_QUBITS, dtype=np.float64)
    for (i, j), th in zip(GATES, np.asarray(angles, dtype=np.float64)):
        c, s = np.cos(th), np.sin(th)
        Ri, Rj = R[i].copy(), R[j].copy()
        R[i] = c * Ri + s * Rj
        R[j] = -s * Ri + c * Rj
    A = np.asarray(pairs)
    a_, b_ = A[:, 0], A[:, 1]
    M = (R[np.ix_(a_, a_)] * R[np.ix_(b_, b_)]
         - R[np.ix_(a_, b_)] * R[np.ix_(b_, a_)])  # M[k, l]
    W = M.T[np.ix_(lperm, kperm)]  # W_used[l, k], permuted
    blocks = []
    for j in range(4):
        rows = W[LSPLIT[j] : LSPLIT[j + 1], : nj[j]]
        blk = np.zeros((ROWS[j], nj[j]), dtype=np.float64)
        blk[: rows.shape[0]] = rows
        blocks.append(blk)
    return blocks, lperm, kperm, nj


def _build_module(precision=DEFAULT_PRECISION):
    import concourse.bacc as bacc
    import concourse.mybir as mybir
    from concourse.tile import TileContext

    _, _, _, nj = _orderings()
    bf16 = mybir.dt.bfloat16
    f32 = mybir.dt.float32
    nc = bacc.Bacc("TRN2", target_bir_lowering=False, debug=False)

    rest = nj[1] - C1SPLIT  # w-chunk-1 tail columns, merged into the 0w piece
    piece_shapes = {
        "3": (ROWS[3], nj[3] + BS),
        "2": (ROWS[2], nj[2] + BS),
        "1a": (ROWS[1], C1SPLIT + 256),
        "0wb": (128, nj[0] + rest),
        "0x01": (ROWS[0], 256),
        "0x23": (ROWS[0], 512),
    }
    in_assign = (("sync", "3"), ("gpsimd", "2"), ("sync", "1a"),
                 ("sync", "0wb"), ("gpsimd", "0x01"), ("sync", "0x23"))
    drt = {k: nc.dram_tensor(f"in{k}", list(s), bf16, kind="ExternalInput").ap()
           for k, s in piece_shapes.items()}
    y = nc.dram_tensor("y", [BS, D], bf16, kind="ExternalOutput").ap()
    eng = {"sync": nc.sync, "scalar": nc.scalar, "vector": nc.vector,
           "gpsimd": nc.gpsimd}

    with TileContext(nc) as tc:
        with (
            tc.tile_pool(name="const", bufs=1) as cpool,
            tc.tile_pool(name="yout", bufs=1) as ypool,
            tc.tile_pool(name="psy", bufs=1, space="PSUM") as psy,
        ):
            ptiles = {}
            for e_name, key in in_assign:
                t = cpool.tile(list(piece_shapes[key]), bf16,
                               tag=f"p{key}", name=f"p{key}")
                ptiles[key] = t
                eng[e_name].dma_start(t[:], drt[key])

            # PE warmup: establishes pe_busy_start early so real waves,
            # visited after their piece sems (>3.7us), are charged warm.
            scratch = cpool.tile([128, 128], bf16, tag="scr", name="scr")
            nc.vector.memset(scratch[:], 1.0)
            wps = psy.tile([128, 128], f32, tag="wps", name="wps")
            for _ in range(3):
                nc.tensor.matmul(wps[:], lhsT=scratch[:, 0:128],
                                 rhs=scratch[:], start=True, stop=True)
            # preload ACT tables off the critical path
            asc = cpool.tile([1, 8], f32, tag="asc", name="asc")
            nc.vector.memset(asc[:], 0.0)
            ad = cpool.tile([1, 8], bf16, tag="ad", name="ad")
            nc.scalar.copy(ad[:], asc[:])

            pairs = [psy.tile([128, 1024], f32, tag=f"pp{i}", name=f"pp{i}")
                     for i in range(2)]

            def ps(g, lo, hi):
                pi, s = divmod(g, 2)
                return pairs[pi][:, s * 512 + lo : s * 512 + hi]

            for wi, key in enumerate(("3", "2")):
                j = int(key)
                t = ptiles[key]
                for g in range(4):
                    nc.tensor.matmul(
                        ps(g, 0, nj[j]),
                        lhsT=t[0 : ROWS[j], nj[j] + g * 128 : nj[j] + (g + 1) * 128],
                        rhs=t[0 : ROWS[j], 0 : nj[j]],
                        start=(wi == 0), stop=False,
                    )
            # chunk-1 x is group-split too: g0/g1 halves ride in piece 1a,
            # g2/g3 halves ride (deferred) in piece 0x23 cols 256:512
            t1a = ptiles["1a"]
            t23 = ptiles["0x23"]
            for g in (0, 1):
                nc.tensor.matmul(
                    ps(g, 0, C1SPLIT),
                    lhsT=t1a[0 : ROWS[1], C1SPLIT + g * 128 : C1SPLIT + (g + 1) * 128],
                    rhs=t1a[0 : ROWS[1], 0:C1SPLIT],
                    start=False, stop=False,
                )
            # tail: groups 0/1 fully finish (c1b, c0), then groups 2/3
            t0w = ptiles["0wb"]
            tx = {0: ptiles["0x01"], 1: ptiles["0x01"],
                  2: ptiles["0x23"], 3: ptiles["0x23"]}

            def x1_lhs(g):
                if g < 2:
                    return t1a[0 : ROWS[1],
                               C1SPLIT + g * 128 : C1SPLIT + (g + 1) * 128]
                return t23[0 : ROWS[1],
                           256 + (g % 2) * 128 : 256 + (g % 2 + 1) * 128]

            for gs in ((0, 1), (2, 3)):
                if gs == (2, 3):
                    for g in gs:
                        nc.tensor.matmul(
                            ps(g, 0, C1SPLIT),
                            lhsT=x1_lhs(g),
                            rhs=t1a[0 : ROWS[1], 0:C1SPLIT],
                            start=False, stop=False,
                        )
                for g in gs:
                    nc.tensor.matmul(
                        ps(g, C1SPLIT, nj[1]),
                        lhsT=x1_lhs(g),
                        rhs=t0w[0 : ROWS[1], nj[0] : nj[0] + rest],
                        start=False, stop=False,
                    )
                for g in gs:
                    xo = (g % 2) * 128
                    nc.tensor.matmul(
                        ps(g, 0, nj[0]),
                        lhsT=tx[g][0 : ROWS[0], xo : xo + 128],
                        rhs=t0w[0 : ROWS[0], 0 : nj[0]],
                        start=False, stop=True,
                    )

            ysp = [ypool.tile([128, 2 * D], bf16, tag=f"ysp{i}", name=f"ysp{i}")
                   for i in range(2)]
            for pi, e_name in enumerate(("vector", "scalar")):
                src = pairs[pi][:].rearrange("p (g k) -> p g k", g=2)[:, :, 0:D]
                dst = ysp[pi][:].rearrange("p (g k) -> p g k", g=2)
                if e_name == "scalar":
                    eng[e_name].copy(dst, src)
                else:
                    eng[e_name].tensor_copy(dst, src)
            for pi in range(2):
                nc.sync.dma_start(
                    y[pi * 256 : (pi + 1) * 256, :].rearrange(
                        "(g b) k -> b g k", g=2),
                    ysp[pi][:].rearrange("b (g k) -> b g k", g=2),
                )
    nc.compile()
    return nc


def _prep_inputs(input_state, angles):
    import ml_dtypes

    blocks, lperm, kperm, nj = _host_weight_blocks(angles)
    x = np.asarray(input_state, dtype=np.float64)
    xp = x[:, lperm]
    in_maps = []
    for c in range(NCORES):
        xc = xp[c * BS : (c + 1) * BS]
        m = {}
        for j, key in ((3, "3"), (2, "2")):
            r = ROWS[j]
            piece = np.zeros((r, nj[j] + BS), dtype=np.float64)
            piece[:, : nj[j]] = blocks[j]
            piece[: LSPLIT[j + 1] - LSPLIT[j], nj[j] :] = (
                xc[:, LSPLIT[j] : LSPLIT[j + 1]].T)
            m[f"in{key}"] = piece.astype(ml_dtypes.bfloat16)
        r = ROWS[1]
        x1 = xc[:, LSPLIT[1] : LSPLIT[2]].T  # [128, 512]
        pa = np.zeros((r, C1SPLIT + 256), dtype=np.float64)
        pa[:, :C1SPLIT] = blocks[1][:, :C1SPLIT]
        pa[:r, C1SPLIT:] = x1[:, 0:256]
        m["in1a"] = pa.astype(ml_dtypes.bfloat16)
        m["in0wb"] = np.concatenate(
            [blocks[0], blocks[1][:, C1SPLIT:]], axis=1).astype(ml_dtypes.bfloat16)
        x0 = xc[:, LSPLIT[0] : LSPLIT[1]].T  # [128, 512]
        m["in0x01"] = np.ascontiguousarray(x0[:, 0:256]).astype(ml_dtypes.bfloat16)
        m["in0x23"] = np.concatenate(
            [x0[:, 256:512], x1[:, 256:512]], axis=1).astype(ml_dtypes.bfloat16)
        in_maps.append(m)
    return in_maps, kperm


def run_device(input_state, angles, trace=False, precision=DEFAULT_PRECISION,
               **trace_kw):
    """Shard, run on 8 cores, gather. Returns (out, BassKernelResults)."""
    if precision not in _NC:
        _NC[precision] = _build_module(precision)
    from concourse import bass_utils

    in_maps, kperm = _prep_inputs(input_state, angles)
    res = bass_utils.run_bass_kernel_spmd(
        _NC[precision], in_maps, core_ids=list(range(NCORES)), trace=trace,
        **trace_kw
    )
    out = np.concatenate(
        [np.asarray(res.results[c]["y"], dtype=np.float32)
         for c in range(NCORES)], axis=0)
    inv = np.argsort(kperm)
    out = np.ascontiguousarray(out[:, inv])
    return out, res


def kernel(input_state, angles, U=None, **_ignored) -> np.ndarray:
    out, _ = run_device(input_state, angles, trace=False)
    return out
